# revision 25
# baseline (speedup 1.0000x reference)
"""Trainium2 Bass kernel for the additive coupling flow (nn_Additive_flow).

Math: 65 sequential steps. Step s (i = idx[s]) updates column i of z:
    z[:, i] += MLP_s(z with cols i<->63 swapped, first 63 cols) + b3[s]
Reformulated so no data permutation is ever needed on device:
    h1 = relu(z @ W1e[s] + b1[s])      W1e[s] = [W1[s]; 0] with rows i,63 swapped
    h2 = relu(h1 @ W2[s] + b2[s])
    z += h2 @ W3e[s] + b3c[:, s]       W3e[s] = w3[s] scattered into column i
Finally out = exp(s_vec) * z.

Device layout: z kept transposed ([64 features, batch]) in SBUF for the whole
kernel; all matmul operands are bf16 (full-rate on the PE), fp32 PSUM
accumulation. Data-parallel over 8 cores on the batch dim; no collectives.
"""

import os
import sys

for _p in ("/opt/trn_rl_repo", "/root/.axon_site/_ro/trn_rl_repo"):
    if os.path.isdir(_p) and _p not in sys.path:
        sys.path.append(_p)

import numpy as np
import concourse.bass as bass
import concourse.bacc as bacc
import concourse.mybir as mybir
from concourse.tile import TileContext
from concourse.bass_utils import run_bass_kernel_spmd

NCORES = 8
B = 131072
N = 64          # latent dim
S = 65          # coupling steps
H = 256         # MLP width
BSH = B // NCORES      # 16384 samples per core
TILE = 512             # matmul moving free-dim (max for 4-byte dtypes)
MACRO = 1024           # elementwise tile (2 matmul tiles share one ACT/DVE op)

F32 = mybir.dt.float32
F32R = mybir.dt.float32r
BF16 = mybir.dt.bfloat16
AF = mybir.ActivationFunctionType
ALU = mybir.AluOpType

LAST_RESULT = None  # test.py reads exec_time_ns from here


def build_program(nsteps=S, nmacro=BSH // MACRO, use_bias=False, hbufs=None):
    if hbufs is None:
        hbufs = int(os.environ.get("KERNEL_HBUFS", "3"))
    bsh = nmacro * MACRO
    nc = bacc.Bacc("TRN2", target_bir_lowering=False, debug=False)

    xt = nc.dram_tensor("xt", [N, bsh], BF16, kind="ExternalInput")
    # all weights for one step packed into a single [128, 896] DMA:
    # cols 0:256 w1e (partitions 0:64), 256:512 w2 k-chunk a, 512:768 w2
    # k-chunk b, 768:832 w3e chunk a, 832:896 w3e chunk b
    wp_d = nc.dram_tensor("wpack", [nsteps, 128, 896], BF16, kind="ExternalInput")
    b1_d = nc.dram_tensor("b1r", [128, 2 * nsteps], F32, kind="ExternalInput")
    b2_d = nc.dram_tensor("b2r", [128, 2 * nsteps], F32, kind="ExternalInput")
    b3_d = nc.dram_tensor("b3c", [N, nsteps], F32, kind="ExternalInput")
    s_d = nc.dram_tensor("sv", [N, 1], F32, kind="ExternalInput")
    out_d = nc.dram_tensor("out", [N, bsh], F32, kind="ExternalOutput")

    with TileContext(nc) as tc:
        with (
            tc.tile_pool(name="zpool", bufs=1) as zp,
            tc.tile_pool(name="consts", bufs=1) as cp,
            tc.tile_pool(name="wpool", bufs=4) as wp,
            tc.tile_pool(name="hpool", bufs=hbufs) as hp,
            tc.tile_pool(name="psA", bufs=3, space="PSUM") as pA,
            tc.tile_pool(name="psB", bufs=3, space="PSUM") as pB,
            tc.tile_pool(name="psZ", bufs=2, space="PSUM") as pZ,
        ):
            # --- constants, loaded once ---
            if use_bias:
                b1s = cp.tile([128, 2 * nsteps], F32, tag="b1s")
                nc.sync.dma_start(b1s[:], b1_d[:])
                b2s = cp.tile([128, 2 * nsteps], F32, tag="b2s")
                nc.sync.dma_start(b2s[:], b2_d[:])
                b3s = cp.tile([N, nsteps], F32, tag="b3s")
                nc.sync.dma_start(b3s[:], b3_d[:])
            ss = cp.tile([N, 1], F32, tag="ss")
            nc.sync.dma_start(ss[:], s_d[:])
            exps = cp.tile([N, 1], F32, tag="exps")
            nc.scalar.activation(exps[:], ss[:], AF.Exp)

            def fetch_weights(st):
                wt = wp.tile([128, 896], BF16, tag="w")
                nc.sync.dma_start(wt[:], wp_d[st])
                w1t = wt[0:N, 0:H]
                w2ta = wt[:, 256:512]
                w2tb = wt[:, 512:768]
                w3ta = wt[:, 768:832]
                w3tb = wt[:, 832:896]
                return w1t, w2ta, w2tb, w3ta, w3tb

            wtiles = fetch_weights(0)

            # --- z state, resident in SBUF, feature-major [64, bsh] ---
            # loaded on the gpsimd DMA queue so it doesn't delay the
            # weight stream on the sync queue
            zt = zp.tile([N, bsh], BF16, tag="z")
            for m in range(nmacro):
                msl = bass.ts(m, MACRO)
                nc.gpsimd.dma_start(zt[:, msl], xt[:, msl])

            # layer-3 of macro-tile m is deferred until after layer-1 of
            # macro-tile m+1 so the PE always has ready work while the
            # scalar engine runs the layer-1 relus (1-stage SW pipeline)
            pending_l3 = None

            for st in range(nsteps):
                if st > 0:
                    wtiles = fetch_weights(st)
                w1t, w2ta, w2tb, w3ta, w3tb = wtiles

                for m in range(nmacro):
                    zsl = zt[:, bass.ts(m, MACRO)]

                    def act_relu(out, in_, bcol):
                        if use_bias:
                            nc.scalar.activation(out, in_, AF.Relu, bias=bcol)
                        else:
                            nc.scalar.activation(out, in_, AF.Relu)

                    def dve_relu(out, in_, bcol):
                        if use_bias:
                            nc.vector.tensor_scalar(
                                out, in_, bcol, 0.0, op0=ALU.add, op1=ALU.max
                            )
                        else:
                            nc.vector.tensor_scalar(out, in_, 0.0, None, op0=ALU.max)

                    b1a = b1s[:, 2 * st : 2 * st + 1] if use_bias else None
                    b1b = b1s[:, 2 * st + 1 : 2 * st + 2] if use_bias else None
                    b2a = b2s[:, 2 * st : 2 * st + 1] if use_bias else None
                    b2b = b2s[:, 2 * st + 1 : 2 * st + 2] if use_bias else None

                    # ---- layer 1: h1 = relu(W1e.T @ z + b1) ----
                    # per-512 psum tiles so dependency tracking stays
                    # half-granular (no same-tensor write/read serialization)
                    h1ps = []
                    for t in range(MACRO // TILE):
                        tsl = bass.ts(t, TILE)
                        pa = pA.tile([128, TILE], F32, tag="h1p")
                        pb = pA.tile([128, TILE], F32, tag="h1p")
                        nc.tensor.matmul(pa[:], w1t[:, 0:128], zsl[:, tsl])
                        nc.tensor.matmul(pb[:], w1t[:, 128:256], zsl[:, tsl])
                        h1ps.append((pa, pb))
                    if pending_l3 is not None:
                        pending_l3()
                        pending_l3 = None
                    h1a = hp.tile([128, MACRO], BF16, tag="h1a")
                    h1b = hp.tile([128, MACRO], BF16, tag="h1b")
                    # h0 relus run in parallel on ACT + DVE
                    act_relu(h1a[:, 0:TILE], h1ps[0][0][:], b1a)
                    dve_relu(h1b[:, 0:TILE], h1ps[0][1][:], b1b)
                    act_relu(h1a[:, TILE:MACRO], h1ps[1][0][:], b1a)
                    act_relu(h1b[:, TILE:MACRO], h1ps[1][1][:], b1b)

                    # ---- layer 2: h2 = relu(W2.T @ h1 + b2) ----
                    # the deferred layer-3 halves of macro m-1 are interleaved
                    # between the matmul groups so every sync-waiting group
                    # has independent PE work queued right before it
                    h2a = hp.tile([128, MACRO], BF16, tag="h2a")
                    h2b = hp.tile([128, MACRO], BF16, tag="h2b")
                    for t in range(MACRO // TILE):
                        tsl = bass.ts(t, TILE)
                        pa = pB.tile([128, TILE], F32, tag="h2p")
                        pb = pB.tile([128, TILE], F32, tag="h2p")
                        nc.tensor.matmul(
                            pa[:], w2ta[:, 0:128], h1a[:, tsl], start=True, stop=False
                        )
                        nc.tensor.matmul(
                            pa[:], w2tb[:, 0:128], h1b[:, tsl], start=False, stop=True
                        )
                        nc.tensor.matmul(
                            pb[:], w2ta[:, 128:256], h1a[:, tsl], start=True, stop=False
                        )
                        nc.tensor.matmul(
                            pb[:], w2tb[:, 128:256], h1b[:, tsl], start=False, stop=True
                        )
                        act_relu(h2a[:, tsl], pa[:], b2a)
                        dve_relu(h2b[:, tsl], pb[:], b2b)

                    # ---- layer 3 + state update: z += W3e.T @ h2 + b3c ----
                    def emit_l3(h2a=h2a, h2b=h2b, zsl=zsl, w3ta=w3ta, w3tb=w3tb, st=st):
                        for t in range(MACRO // TILE):
                            tsl = bass.ts(t, TILE)
                            zps = pZ.tile([N, TILE], F32, tag="zp")
                            nc.tensor.matmul(
                                zps[:], w3ta[:], h2a[:, tsl], start=True, stop=False
                            )
                            nc.tensor.matmul(
                                zps[:], w3tb[:], h2b[:, tsl], start=False, stop=True
                            )
                            ztile = zsl[:, tsl]
                            if use_bias:
                                nc.vector.scalar_tensor_tensor(
                                    ztile,
                                    zps[:],
                                    b3s[:, st : st + 1],
                                    ztile,
                                    op0=ALU.add,
                                    op1=ALU.add,
                                )
                            else:
                                nc.vector.tensor_add(ztile, zps[:], ztile)

                    pending_l3 = emit_l3

            if pending_l3 is not None:
                pending_l3()
                pending_l3 = None

            # --- final scale + store (f32 staging for full output precision) ---
            for m in range(nmacro):
                msl = bass.ts(m, MACRO)
                ostage = hp.tile([N, MACRO], F32, tag="ostage")
                nc.vector.tensor_scalar_mul(ostage[:], zt[:, msl], exps[:])
                nc.sync.dma_start(out_d[:, msl], ostage[:])

    nc.finalize()
    return nc


def host_prep(x, s, W1, b1, W2, b2, W3, b3, idx, nsteps=S):
    """Build the device-side parameter arrays (all tiny except x transpose)."""
    x = np.asarray(x, np.float32)
    idx = np.asarray(idx)
    W1 = np.asarray(W1, np.float32)
    W2 = np.ascontiguousarray(np.asarray(W2, np.float32)[:nsteps])
    W3 = np.asarray(W3, np.float32)
    b1 = np.asarray(b1, np.float32)
    b2 = np.asarray(b2, np.float32)
    b3 = np.asarray(b3, np.float32)

    W1e = np.zeros((nsteps, N, H), np.float32)
    W1e[:, : N - 1, :] = W1[:nsteps]
    for st in range(nsteps):
        i = int(idx[st])
        r = W1e[st].copy()
        r[[i, N - 1]] = r[[N - 1, i]]
        W1e[st] = r
    W3e = np.zeros((nsteps, H, N), np.float32)
    for st in range(nsteps):
        W3e[st, :, int(idx[st])] = W3[st, :, 0]
    b3c = np.zeros((N, nsteps), np.float32)
    for st in range(nsteps):
        b3c[int(idx[st]), st] = b3[st, 0]
    import ml_dtypes
    wpack = np.zeros((nsteps, 128, 896), np.float32)
    wpack[:, 0:N, 0:H] = W1e
    wpack[:, :, 256:512] = W2[:, 0:128, :]
    wpack[:, :, 512:768] = W2[:, 128:256, :]
    wpack[:, :, 768:832] = W3e[:, 0:128, :]
    wpack[:, :, 832:896] = W3e[:, 128:256, :]
    # biases as [128, 2*nsteps]: col 2s = b[s][0:128], col 2s+1 = b[s][128:256]
    b1r = np.ascontiguousarray(
        b1[:nsteps].reshape(nsteps, 2, 128).transpose(2, 0, 1).reshape(128, 2 * nsteps)
    )
    b2r = np.ascontiguousarray(
        b2[:nsteps].reshape(nsteps, 2, 128).transpose(2, 0, 1).reshape(128, 2 * nsteps)
    )
    wpack = wpack.astype(ml_dtypes.bfloat16)
    xt = np.ascontiguousarray(x.T).astype(ml_dtypes.bfloat16)  # [64, B]
    sv = np.ascontiguousarray(np.asarray(s, np.float32).reshape(N, 1))
    return dict(wpack=wpack, b1r=b1r, b2r=b2r, b3c=b3c, sv=sv), xt


_PROGRAM = {}


def kernel(x, s, W1, b1, W2, b2, W3, b3, idx):
    global LAST_RESULT
    use_bias = bool(
        np.abs(b1).max() > 0 or np.abs(b2).max() > 0 or np.abs(b3).max() > 0
    )
    shared, xt = host_prep(x, s, W1, b1, W2, b2, W3, b3, idx)
    in_maps = []
    for c in range(NCORES):
        m = dict(shared)
        m["xt"] = np.ascontiguousarray(xt[:, c * BSH : (c + 1) * BSH])
        in_maps.append(m)

    if use_bias not in _PROGRAM:
        _PROGRAM[use_bias] = build_program(use_bias=use_bias)
    _P = _PROGRAM[use_bias]
    res = run_bass_kernel_spmd(_P, in_maps, core_ids=list(range(NCORES)))
    LAST_RESULT = res
    outs = [res.results[c]["out"] for c in range(NCORES)]
    return np.ascontiguousarray(
        np.concatenate([o.T for o in outs], axis=0), dtype=np.float32
    )
